# revision 1
# baseline (speedup 1.0000x reference)
"""AR block LSTM on 8 TRN2 NeuronCores.

Data-parallel over batch (1024 -> 128/core), weights replicated.
Per core, each LSTM step computes z = [x;1] @ [Wx;b] + h @ Uh into 8 PSUM
banks (4096 gate cols, native i|f|g|o order), does the cell math on
ACT/DVE in unit-halves, and transposes h2 back to [units, batch] layout
with PE-transposes so it can serve as the next step's stationary operand.
The AR phase computes pT = (h @ Wd + bd)^T with 8 small matmuls; pT is both
the output block and the next step's x input (so no feedback transpose).

Matmuls run in bf16 (near the 213 ns/MM N=512 PE roofline; end-to-end
rel err ~4e-3 vs the fp32 reference); the cell state c stays fp32.
Half-B transposes are deferred into the next step's matmul stream so
the in-order PE does not stall on the ACT/DVE cell math, and the k=0
matmuls open every accumulation group while the x-packs close it
mid-stream. Measured on silicon: 2.4925 ms whole-NEFF exec, 8 cores.
"""
import os
import numpy as np
import ml_dtypes

import concourse.bass as bass
import concourse.mybir as mybir
import concourse.tile as tile
from concourse import bacc
from concourse.bass_utils import run_bass_kernel_spmd

F32 = mybir.dt.float32
BF16 = mybir.dt.bfloat16

N_CORES = 8
BATCH = 1024
B = BATCH // N_CORES          # 128 batch rows per core
WARM_T = 128
UNITS = 1024
KT = UNITS // 128             # 8 K-tiles
G = 4 * UNITS                 # 4096 gate cols
NBANK = G // 512              # 8 PSUM banks of 512 gate cols
FEAT = 8
NBLK = 192 // FEAT            # 24 output blocks
CH = 16                       # xT chunk length (warmup steps per DMA)

ACT = mybir.ActivationFunctionType


def build(nwarm=WARM_T, nar=NBLK - 1):
    nc = bacc.Bacc("TRN2", target_bir_lowering=False, debug=False,
                   num_devices=N_CORES)
    xt_e = nc.dram_tensor("xt", [nwarm, FEAT + 1, B], BF16, kind="ExternalInput").ap()
    uh_e = nc.dram_tensor("uh", [KT, 128, G], BF16, kind="ExternalInput").ap()
    wxa_e = nc.dram_tensor("wxa", [FEAT + 1, G], BF16, kind="ExternalInput").ap()
    wd_e = nc.dram_tensor("wd", [KT, 128, FEAT], BF16, kind="ExternalInput").ap()
    bd_e = nc.dram_tensor("bdv", [FEAT, 1], F32, kind="ExternalInput").ap()
    id_e = nc.dram_tensor("ident", [128, 128], BF16, kind="ExternalInput").ap()
    out_e = nc.dram_tensor("out", [nar + 1, FEAT, B], F32, kind="ExternalOutput").ap()

    nsteps = nwarm + nar

    with tile.TileContext(nc) as tc:
        with tc.tile_pool(name="w", bufs=1) as wp, \
             tc.tile_pool(name="xt", bufs=2) as xtp, \
             tc.tile_pool(name="ht", bufs=2) as htp, \
             tc.tile_pool(name="st", bufs=1) as stp, \
             tc.tile_pool(name="h2", bufs=2) as h2p, \
             tc.tile_pool(name="gate", bufs=6) as gp, \
             tc.tile_pool(name="m", bufs=4) as mp, \
             tc.tile_pool(name="pt", bufs=2) as ptp, \
             tc.tile_pool(name="z", bufs=6, space="PSUM") as zp, \
             tc.tile_pool(name="tr", bufs=2, space="PSUM") as trp:

            # ---- resident weights -------------------------------------
            uh_sb = []
            for k in range(KT):
                u1 = wp.tile([128, G], BF16, tag=f"uh{k}")
                nc.sync.dma_start(out=u1[:], in_=uh_e[k])
                uh_sb.append(u1)
            # Wx+bias rows replicated at partitions 0/32/64/96 for row-packed
            # K=9 matmuls via tile_position.
            wxa_sb = wp.tile([128, G], BF16)
            for r in range(4):
                nc.sync.dma_start(out=wxa_sb[32 * r:32 * r + FEAT + 1, :], in_=wxa_e[:])
            wd_sb = wp.tile([128, KT, FEAT], BF16)
            for k in range(KT):
                nc.sync.dma_start(out=wd_sb[:, k, :], in_=wd_e[k])
            bd_sb = wp.tile([FEAT, 1], F32)
            nc.sync.dma_start(out=bd_sb[:], in_=bd_e[:])
            id_sb = wp.tile([128, 128], BF16)
            nc.sync.dma_start(out=id_sb[:], in_=id_e[:])

            # ---- state ------------------------------------------------
            hT = htp.tile([128, KT, B], BF16, tag="hT")
            nc.gpsimd.memset(hT[:], 0.0)
            c_sb = stp.tile([128, UNITS], F32)
            nc.gpsimd.memset(c_sb[:], 0.0)

            # ---- xT chunk prefetch ------------------------------------
            nchunk = (nwarm + CH - 1) // CH
            chunk_tiles = {}

            def load_chunk(ci):
                t0 = ci * CH
                n = min(CH, nwarm - t0)
                tl = xtp.tile([128, CH, B], BF16, tag="xt")
                for r in range(4):
                    nc.sync.dma_start(
                        out=tl[32 * r:32 * r + FEAT + 1, :n, :],
                        in_=xt_e[t0:t0 + n].rearrange("t p b -> p t b"))
                return tl

            if nwarm > 0:
                chunk_tiles[0] = load_chunk(0)

            def p_block(j, hT_new):
                """pT = (h_new @ Wd + bd)^T -> out block j; returns pT_aug."""
                pp = trp.tile([FEAT, B], F32, tag="tr")
                for k in range(KT):
                    nc.tensor.matmul(pp[:], wd_sb[:, k, :], hT_new[:, k, :],
                                     start=(k == 0), stop=(k == KT - 1))
                pto = ptp.tile([FEAT, B], F32, tag="pto")
                nc.scalar.activation(pto[:], pp[:], ACT.Identity, bias=bd_sb[:])
                nc.sync.dma_start(out=out_e[j], in_=pto[:])
                pta = ptp.tile([128, B], BF16, tag="pta")
                nc.gpsimd.memset(pta[:], 1.0)
                for r in range(4):
                    nc.vector.tensor_copy(pta[32 * r:32 * r + FEAT, :], pto[:])
                return pta

            pta = None
            pending_tr = None
            bank_order = [0, 2, 4, 6, 1, 3, 5, 7]

            for t in range(nsteps):
                warm = t < nwarm
                if warm:
                    ci, s = divmod(t, CH)
                    if s == 0 and ci + 1 < nchunk:
                        chunk_tiles[ci + 1] = load_chunk(ci + 1)
                        chunk_tiles.pop(ci - 1, None)
                    ct = chunk_tiles[ci]
                    xaug = lambda r: ct[32 * r:32 * r + FEAT + 1, s, :]
                else:
                    cpta = pta
                    xaug = lambda r: cpta[32 * r:32 * r + FEAT + 1, :]

                # ---- z matmuls, bank-major so early banks finish early
                zt = {}
                h2 = h2p.tile([128, UNITS], BF16, tag="h2")
                hT_new = htp.tile([128, KT, B], BF16, tag="hT")

                def half_chain(h):
                    u0 = h * 512
                    si = gp.tile([128, 512], F32, tag="g")
                    sf = gp.tile([128, 512], F32, tag="g")
                    tg = gp.tile([128, 512], F32, tag="g")
                    so = gp.tile([128, 512], F32, tag="g")
                    nc.scalar.activation(si[:], zt[0 + h][:], ACT.Sigmoid)
                    nc.scalar.activation(sf[:], zt[2 + h][:], ACT.Sigmoid)
                    nc.scalar.activation(tg[:], zt[4 + h][:], ACT.Tanh)
                    nc.scalar.activation(so[:], zt[6 + h][:], ACT.Sigmoid)
                    m1 = mp.tile([128, 512], F32, tag="m")
                    m2 = mp.tile([128, 512], F32, tag="m")
                    nc.vector.tensor_mul(m1[:], sf[:], c_sb[:, u0:u0 + 512])
                    nc.vector.tensor_mul(m2[:], si[:], tg[:])
                    nc.vector.tensor_add(c_sb[:, u0:u0 + 512], m1[:], m2[:])
                    tc2 = gp.tile([128, 512], F32, tag="g")
                    nc.scalar.activation(tc2[:], c_sb[:, u0:u0 + 512], ACT.Tanh)
                    nc.vector.tensor_mul(h2[:, u0:u0 + 512], so[:], tc2[:])

                def emit_tr(h, h2t, hTn):
                    # two k-tiles per PSUM scratch tile + one double-width
                    # copy: halves the slot-recycle stalls between transposes
                    for kp in range(2):
                        k0 = 4 * h + 2 * kp
                        tr = trp.tile([128, 2, 128], BF16, tag="tr")
                        nc.tensor.transpose(tr[:, 0, :],
                                            h2t[:, k0 * 128:(k0 + 1) * 128],
                                            id_sb[:])
                        nc.tensor.transpose(tr[:, 1, :],
                                            h2t[:, (k0 + 1) * 128:(k0 + 2) * 128],
                                            id_sb[:])
                        nc.vector.tensor_copy(hTn[:, k0:k0 + 2, :], tr[:])

                def kmms(bk, ks, first_start):
                    n0 = bk * 512
                    for k in ks:
                        nc.tensor.matmul(zt[bk][:], hT[:, k, :],
                                         uh_sb[k][:, n0:n0 + 512],
                                         start=(first_start and k == ks[0]),
                                         stop=False)

                def packs(banks, first):
                    # Row-packed x/bias matmuls: K=9 tiles at PE rows
                    # 0/32/64/96 run concurrently, each into its own PSUM
                    # bank. During warmup x is ready early, so they open the
                    # accumulation groups; in the AR phase x (= pT) arrives
                    # late, so they close them instead.
                    for r, bk in enumerate(banks):
                        n0r = bk * 512
                        nc.tensor.matmul(
                            zt[bk][:],
                            xaug(r),
                            wxa_sb[32 * r:32 * r + FEAT + 1, n0r:n0r + 512],
                            start=first, stop=(t == 0 or not first),
                            tile_position=(32 * r, 0))

                for bk in bank_order:
                    ztile = zp.tile([128, 512], F32, tag="z")
                    zt[bk] = ztile
                bA, bB = bank_order[:4], bank_order[4:]

                if pending_tr is not None:
                    # Finish the previous step's half-B transposes in the
                    # middle of this step's half-A matmuls: k-tiles 0..3 for
                    # all four banks (~3.4us of PE work) only need half-A
                    # k-tiles, which covers the cell-math latency of the
                    # previous half-B chain so the PE never stalls.
                    assert warm and t > 0
                    for bk in bA:
                        kmms(bk, [0, 1, 2, 3], True)
                    pending_tr()
                    pending_tr = None
                    for bk in bA:
                        kmms(bk, [4, 5, 6, 7], False)
                    packs(bA, first=False)
                elif warm:
                    # only t == 0 (no k matmuls): the packs are the whole group
                    packs(bA, first=True)
                else:
                    for bk in bA:
                        kmms(bk, list(range(KT)), True)
                    packs(bA, first=False)
                half_chain(0)

                if warm and t == 0:
                    packs(bB, first=True)
                else:
                    for bk in bB:
                        kmms(bk, list(range(KT)), True)
                    packs(bB, first=False)
                emit_tr(0, h2, hT_new)
                half_chain(1)

                if warm and t < nwarm - 1:
                    # defer half-B transposes into the next step's stream
                    ch2, chT = h2, hT_new
                    pending_tr = lambda: emit_tr(1, ch2, chT)
                else:
                    emit_tr(1, h2, hT_new)

                hT = hT_new
                if t >= nwarm - 1:
                    pta = p_block(t - nwarm + 1, hT_new)

    nc.finalize()
    return nc


_NC_CACHE = {}


def _get_nc(nwarm, nar):
    key = (nwarm, nar)
    if key not in _NC_CACHE:
        _NC_CACHE[key] = build(nwarm, nar)
    return _NC_CACHE[key]


def prep_inputs(inputs, Wx, Uh, b, Wd, bd, nwarm=WARM_T):
    """Host-side prep: shard + transpose + bf16. Returns in_maps list."""
    bf = ml_dtypes.bfloat16
    uh = np.ascontiguousarray(
        Uh.astype(np.float32).reshape(KT, 128, G)).astype(bf)
    wxa = np.concatenate(
        [Wx.astype(np.float32), b.astype(np.float32)[None, :]], axis=0).astype(bf)
    wd = np.ascontiguousarray(
        Wd.astype(np.float32).reshape(KT, 128, FEAT)).astype(bf)
    bdv = np.ascontiguousarray(bd.astype(np.float32).reshape(FEAT, 1))
    ident = np.eye(128, dtype=np.float32).astype(bf)

    in_maps = []
    for ci in range(N_CORES):
        shard = np.asarray(inputs[ci * B:(ci + 1) * B, :nwarm, :], dtype=np.float32)
        # [B, T, F] -> [T, F, B], then append the ones row -> [T, F+1, B]
        xt = np.transpose(shard, (1, 2, 0))
        xt = np.concatenate([xt, np.ones((nwarm, 1, B), np.float32)], axis=1)
        in_maps.append({
            "xt": np.ascontiguousarray(xt).astype(bf),
            "uh": uh, "wxa": wxa, "wd": wd, "bdv": bdv, "ident": ident,
        })
    return in_maps


def run(inputs, Wx, Uh, b, Wd, bd, nwarm=WARM_T, nar=NBLK - 1, trace=False):
    nc = _get_nc(nwarm, nar)
    in_maps = prep_inputs(inputs, Wx, Uh, b, Wd, bd, nwarm)
    res = run_bass_kernel_spmd(nc, in_maps, core_ids=list(range(N_CORES)),
                               trace=trace)
    outs = []
    for ci in range(N_CORES):
        o = res.results[ci]["out"]          # [nblk, FEAT, B]
        outs.append(np.transpose(o, (2, 0, 1)).reshape(B, (nar + 1) * FEAT, 1))
    full = np.concatenate(outs, axis=0).astype(np.float32)
    return full, res


def kernel(inputs, Wx, Uh, b, Wd, bd):
    full, _ = run(np.asarray(inputs), np.asarray(Wx), np.asarray(Uh),
                  np.asarray(b), np.asarray(Wd), np.asarray(bd))
    return full


if __name__ == "__main__":
    rng = np.random.default_rng(0)
    s = 0.05
    inputs = rng.standard_normal((BATCH, WARM_T, FEAT)).astype(np.float32)
    Wx = (rng.standard_normal((FEAT, G)) * s).astype(np.float32)
    Uh = (rng.standard_normal((UNITS, G)) * s).astype(np.float32)
    b = np.zeros(G, np.float32)
    Wd = (rng.standard_normal((UNITS, FEAT)) * s).astype(np.float32)
    bd = np.zeros(FEAT, np.float32)
    out = kernel(inputs=inputs, Wx=Wx, Uh=Uh, b=b, Wd=Wd, bd=bd)
    print("out shape:", out.shape, out.dtype)



# revision 17
# speedup vs baseline: 1.3766x; 1.3766x over previous
"""AR block LSTM on 8 TRN2 NeuronCores.

Data-parallel over batch (1024 -> 128/core), weights replicated.
Per core, each LSTM step computes z = [x;1] @ [Wx;b] + h @ Uh into 8 PSUM
banks (4096 gate cols, native i|f|g|o order), does the cell math on
ACT/DVE in unit-halves, and transposes h2 back to [units, batch] layout
with PE-transposes so it can serve as the next step's stationary operand.

Precision schedule: the first T8=112 warmup steps run the h @ Uh matmuls
in fp8e4 DoubleRow mode (two k-tiles per matmul, measured 2x PE
throughput); the remaining warmup + all AR steps run bf16.  CPU
simulation shows the LSTM forget gates wash out early-step fp8
quantization noise: end-to-end max-rel error stays near the all-bf16
level as long as the last ~40 steps are bf16.  No operand scaling is
needed at these magnitudes.  Gates and the cell state c are bf16 (DVE
tensor ops hit the 2x packed mode; simulated end-to-end error ~6e-3 vs
the 2e-2 gate); PSUM accumulation stays fp32.

Scheduling: in warm steps the x/bias row-packed matmuls OPEN each PSUM
accumulation group (x is ready early) so each bank closes at its last
k-matmul and the ACT/DVE cell chain starts as early as possible; banks
are processed in chain-dependency order i,g,f,o.  Half-B transposes are
deferred into the next step's matmul stream (pending_tr) for ALL steps
incl. AR, and in the AR phase the p-block (output head + feedback) is
also deferred into the next step's stream, hiding the whole feedback
chain under the next step's k-matmuls.  PSUM: 7 z banks + 1 tr bank.
"""
import os
import numpy as np
import ml_dtypes

import concourse.bass as bass
import concourse.mybir as mybir
import concourse.tile as tile
from concourse import bacc
from concourse.bass_utils import run_bass_kernel_spmd

F32 = mybir.dt.float32
BF16 = mybir.dt.bfloat16
F8 = mybir.dt.float8e4
DR = mybir.MatmulPerfMode.DoubleRow

N_CORES = 8
BATCH = 1024
B = BATCH // N_CORES          # 128 batch rows per core
WARM_T = 128
UNITS = 1024
KT = UNITS // 128             # 8 K-tiles
KP = KT // 2                  # 4 K-tile pairs (DoubleRow)
G = 4 * UNITS                 # 4096 gate cols
NBANK = G // 512              # 8 PSUM banks of 512 gate cols
FEAT = 8
NBLK = 192 // FEAT            # 24 output blocks
CH = 16                       # xT chunk length (warmup steps per DMA)
T8 = 116                      # steps [1, T8) use fp8 DoubleRow k-matmuls

ACT = mybir.ActivationFunctionType


def build(nwarm=WARM_T, nar=NBLK - 1, t8=T8, absorb=True):
    nc = bacc.Bacc("TRN2", target_bir_lowering=False, debug=False,
                   num_devices=N_CORES)
    xt_e = nc.dram_tensor("xt", [nwarm, FEAT + 1, B], BF16, kind="ExternalInput").ap()
    uh_e = nc.dram_tensor("uh", [KT, 128, G], BF16, kind="ExternalInput").ap()
    uh8_e = nc.dram_tensor("uh8", [KP, 128, 2, G], F8, kind="ExternalInput").ap()
    uh2_e = (nc.dram_tensor("uh2", [KT, 128, G], BF16, kind="ExternalInput").ap()
             if absorb else None)
    wxa_e = nc.dram_tensor("wxa", [FEAT + 1, G], BF16, kind="ExternalInput").ap()
    wd_e = nc.dram_tensor("wd", [KT, 128, FEAT], BF16, kind="ExternalInput").ap()
    bd_e = nc.dram_tensor("bdv", [FEAT, 1], F32, kind="ExternalInput").ap()
    id_e = nc.dram_tensor("ident", [128, 128], BF16, kind="ExternalInput").ap()
    out_e = nc.dram_tensor("out", [nar + 1, FEAT, B], F32, kind="ExternalOutput").ap()

    nsteps = nwarm + nar

    with tile.TileContext(nc) as tc:
        with tc.tile_pool(name="w", bufs=1) as wp, \
             tc.tile_pool(name="xt", bufs=2) as xtp, \
             tc.tile_pool(name="ht", bufs=2) as htp, \
             tc.tile_pool(name="h8", bufs=2) as h8p, \
             tc.tile_pool(name="st", bufs=1) as stp, \
             tc.tile_pool(name="h2", bufs=2) as h2p, \
             tc.tile_pool(name="gate", bufs=6) as gp, \
             tc.tile_pool(name="m", bufs=4) as mp, \
             tc.tile_pool(name="pt", bufs=1) as ptp, \
             tc.tile_pool(name="z", bufs=7, space="PSUM") as zp, \
             tc.tile_pool(name="tr", bufs=1, space="PSUM") as trp:

            # ---- resident weights -------------------------------------
            # DMA issue order = priority: the first fp8 step only needs
            # uh8 + chunk0 + wxa; bf16 uh (needed from step T8) and wd
            # (needed from the AR phase) trail.
            wxa_sb = wp.tile([128, G], BF16)
            for r in range(4):
                nc.sync.dma_start(out=wxa_sb[32 * r:32 * r + FEAT + 1, :], in_=wxa_e[:])
            id_sb = wp.tile([128, 128], BF16)
            nc.sync.dma_start(out=id_sb[:], in_=id_e[:])

            # ---- xT chunk prefetch ------------------------------------
            nchunk = (nwarm + CH - 1) // CH
            chunk_tiles = {}

            def load_chunk(ci):
                t0 = ci * CH
                n = min(CH, nwarm - t0)
                tl = xtp.tile([128, CH, B], BF16, tag="xt")
                for r in range(4):
                    nc.sync.dma_start(
                        out=tl[32 * r:32 * r + FEAT + 1, :n, :],
                        in_=xt_e[t0:t0 + n].rearrange("t p b -> p t b"))
                return tl

            if nwarm > 0:
                chunk_tiles[0] = load_chunk(0)

            uh8_sb = []
            for kp in range(KP):
                u8 = wp.tile([128, 2, G], F8, tag=f"uh8{kp}")
                nc.sync.dma_start(out=u8[:], in_=uh8_e[kp])
                uh8_sb.append(u8)

            uh_sb = []
            for k in range(KT):
                u1 = wp.tile([128, G], BF16, tag=f"uh{k}")
                nc.sync.dma_start(out=u1[:], in_=uh_e[k])
                uh_sb.append(u1)
            wd_sb = wp.tile([128, KT, FEAT], BF16)
            for k in range(KT):
                nc.sync.dma_start(out=wd_sb[:, k, :], in_=wd_e[k])
            bd_sb = wp.tile([FEAT, 1], F32)
            nc.sync.dma_start(out=bd_sb[:], in_=bd_e[:])
            uh2_sb = []
            if absorb:
                for k in range(KT):
                    u2 = wp.tile([128, G], BF16, tag=f"uh2{k}")
                    nc.sync.dma_start(out=u2[:], in_=uh2_e[k])
                    uh2_sb.append(u2)

            # ---- state ------------------------------------------------
            hT = h8p.tile([128, KT, B], F8, tag="hT8")
            nc.gpsimd.memset(hT[:], 0.0)
            c_sb = stp.tile([128, UNITS], BF16)
            nc.gpsimd.memset(c_sb[:], 0.0)

            # pta feedback tiles: ones rows written once, p rows per step
            pta_tiles = []
            for i in range(2):
                pt_i = ptp.tile([128, B], BF16, tag=f"pta{i}")
                nc.gpsimd.memset(pt_i[:], 1.0)
                pta_tiles.append(pt_i)

            def p_block(j, hT_new):
                """pT = (h_new @ Wd + bd)^T -> out block j; returns pT_aug."""
                pp = trp.tile([FEAT, B], F32, tag="tr4")
                for k in range(KT):
                    nc.tensor.matmul(pp[:], wd_sb[:, k, :], hT_new[:, k, :],
                                     start=(k == 0), stop=(k == KT - 1))
                pto = ptp.tile([FEAT, B], F32, tag="pto")
                nc.scalar.activation(pto[:], pp[:], ACT.Identity, bias=bd_sb[:])
                nc.sync.dma_start(out=out_e[j], in_=pto[:])
                if absorb:
                    return None
                pta = pta_tiles[j % 2]
                for r in range(4):
                    nc.vector.tensor_copy(pta[32 * r:32 * r + FEAT, :], pto[:])
                return pta

            pta = None
            pend = None       # prev step's half-B transposes
            pend_p = None     # prev step's p-block (AR head + feedback)
            # chain-dependency order: i, g, f, o per half
            bank_order = [0, 4, 2, 6, 1, 5, 3, 7]

            for t in range(nsteps):
                warm = t < nwarm
                use8 = 0 < t < t8
                ab = absorb and not warm
                uh_tbl = uh2_sb if ab else uh_sb
                if warm:
                    ci, s = divmod(t, CH)
                    if s == 0 and ci + 1 < nchunk:
                        chunk_tiles[ci + 1] = load_chunk(ci + 1)
                        chunk_tiles.pop(ci - 1, None)
                    ct = chunk_tiles[ci]
                    xaug = lambda r: ct[32 * r:32 * r + FEAT + 1, s, :]
                else:
                    xaug = lambda r: pta[32 * r:32 * r + FEAT + 1, :]

                zt = {}
                h2 = h2p.tile([128, UNITS], BF16, tag="h2")
                prod8 = (t + 1) < t8   # next step consumes fp8 hT
                if prod8:
                    hT_new = h8p.tile([128, KT, B], F8, tag="hT8")
                else:
                    hT_new = htp.tile([128, KT, B], BF16, tag="hT")

                def half_chain(h):
                    # ACT order matches bank-closing order (i, g, f, o);
                    # DVE ops interleave as their operands become ready.
                    # The half-1 tail (tanh(c), h2 mul) is split in two so
                    # the deferred transposes of k4,5 can start earlier.
                    u0 = h * 512
                    si = gp.tile([128, 512], BF16, tag="g")
                    sf = gp.tile([128, 512], BF16, tag="g")
                    tg = gp.tile([128, 512], BF16, tag="g")
                    so = gp.tile([128, 512], BF16, tag="g")
                    m1 = mp.tile([128, 512], BF16, tag="m")
                    m2 = mp.tile([128, 512], BF16, tag="m")
                    nc.scalar.activation(si[:], zt[0 + h][:], ACT.Sigmoid)
                    nc.scalar.activation(tg[:], zt[4 + h][:], ACT.Tanh)
                    nc.vector.tensor_mul(m2[:], si[:], tg[:])
                    nc.scalar.activation(sf[:], zt[2 + h][:], ACT.Sigmoid)
                    nc.vector.tensor_mul(m1[:], sf[:], c_sb[:, u0:u0 + 512])
                    nc.scalar.activation(so[:], zt[6 + h][:], ACT.Sigmoid)
                    nc.vector.tensor_add(c_sb[:, u0:u0 + 512], m1[:], m2[:])
                    tc2 = gp.tile([128, 512], BF16, tag="g")
                    nc.scalar.activation(tc2[:], c_sb[:, u0:u0 + 512], ACT.Tanh)
                    nc.vector.tensor_mul(h2[:, u0:u0 + 512], so[:], tc2[:])

                def emit_tr(h, h2t, hTn):
                    # 4 transposes into one single-bank PSUM tile, then one
                    # wide copy/cast into the hT layout.  (Splitting this
                    # copy, adding post-pend filler matmuls, or reordering
                    # the consumers all measured SLOWER on silicon: the
                    # changed DVE/PE micro-idle profile trips the HAM clock
                    # gate into 2.4<->1.2 GHz oscillation, ham events 4->60+.)
                    tr = trp.tile([128, 4, 128], BF16, tag="tr4")
                    for kq in range(4):
                        k0 = 4 * h + kq
                        nc.tensor.transpose(tr[:, kq, :],
                                            h2t[:, k0 * 128:(k0 + 1) * 128],
                                            id_sb[:])
                    nc.vector.tensor_copy(hTn[:, 4 * h:4 * h + 4, :], tr[:])

                def kmms(bk, ks, open_grp, close_grp):
                    n0 = bk * 512
                    for j, k in enumerate(ks):
                        nc.tensor.matmul(zt[bk][:], hT[:, k, :],
                                         uh_tbl[k][:, n0:n0 + 512],
                                         start=(open_grp and j == 0),
                                         stop=(close_grp and j == len(ks) - 1))

                def kmms8(bk, kps, open_grp, close_grp):
                    n0 = bk * 512
                    for j, kp in enumerate(kps):
                        nc.tensor.matmul(zt[bk][:], hT[:, 2 * kp:2 * kp + 2, :],
                                         uh8_sb[kp][:, :, n0:n0 + 512],
                                         start=(open_grp and j == 0),
                                         stop=(close_grp and j == len(kps) - 1),
                                         perf_mode=DR)

                def packs(banks, open_grp):
                    # Row-packed x/bias matmuls: K=9 tiles at PE rows
                    # 0/32/64/96 run concurrently, each into its own PSUM
                    # bank. In warm steps x is ready early, so they open
                    # the accumulation groups; in the AR phase x (= pT)
                    # arrives late, so they close them instead.
                    for r, bk in enumerate(banks):
                        n0r = bk * 512
                        nc.tensor.matmul(
                            zt[bk][:],
                            xaug(r),
                            wxa_sb[32 * r:32 * r + FEAT + 1, n0r:n0r + 512],
                            start=open_grp, stop=(t == 0 or not open_grp),
                            tile_position=(32 * r, 0))

                for bk in bank_order:
                    ztile = zp.tile([128, 512], F32, tag="z")
                    zt[bk] = ztile
                bA, bB = bank_order[:4], bank_order[4:]

                # ---- half-A matmul stream --------------------------------
                if warm:
                    packs(bA, open_grp=True)
                if pend is not None:
                    if use8:
                        for bk in bA:
                            kmms8(bk, [0, 1], not warm, False)
                        pend()
                        pend = None
                        for bk in bA:
                            kmms8(bk, [2, 3], False, warm)
                    else:
                        for bk in bA:
                            kmms(bk, [0, 1, 2, 3], not warm, False)
                        pend()
                        pend = None
                        for bk in bA:
                            kmms(bk, [4, 5, 6, 7], False, warm or ab)
                if pend_p is not None:
                    pta = pend_p()
                    pend_p = None
                if not warm and not ab:
                    packs(bA, open_grp=False)
                half_chain(0)

                # ---- half-B matmul stream --------------------------------
                if warm:
                    packs(bB, open_grp=True)
                    if t > 0:
                        if use8:
                            for bk in bB:
                                kmms8(bk, [0, 1, 2, 3], False, True)
                        else:
                            for bk in bB:
                                kmms(bk, list(range(KT)), False, True)
                else:
                    for bk in bB:
                        kmms(bk, list(range(KT)), True, ab)
                    if not ab:
                        packs(bB, open_grp=False)
                emit_tr(0, h2, hT_new)
                half_chain(1)

                if t < nsteps - 1:
                    ch2, chT = h2, hT_new
                    pend = lambda: emit_tr(1, ch2, chT)
                else:
                    emit_tr(1, h2, hT_new)

                hT = hT_new
                if t >= nwarm - 1:
                    if t < nsteps - 1:
                        cj, cht = t - nwarm + 1, hT_new
                        pend_p = lambda: p_block(cj, cht)
                    else:
                        p_block(t - nwarm + 1, hT_new)

    nc.finalize()
    return nc


_NC_CACHE = {}


def _get_nc(nwarm, nar, absorb):
    key = (nwarm, nar, absorb)
    if key not in _NC_CACHE:
        _NC_CACHE[key] = build(nwarm, nar, absorb=absorb)
    return _NC_CACHE[key]


def absorb_ok(Wx, b, bd):
    bp = b.astype(np.float64) + bd.astype(np.float64) @ Wx.astype(np.float64)
    return not np.any(bp)


def prep_inputs(inputs, Wx, Uh, b, Wd, bd, nwarm=WARM_T, absorb=True):
    """Host-side prep: shard + transpose + bf16/fp8. Returns in_maps list."""
    bf = ml_dtypes.bfloat16
    f8 = ml_dtypes.float8_e4m3
    uhf = Uh.astype(np.float32)
    uh = np.ascontiguousarray(uhf.reshape(KT, 128, G)).astype(bf)
    # [KP, 128, 2, G]: uh8[kp][p, i, n] = Uh[(2*kp+i)*128 + p, n]
    uh8 = np.ascontiguousarray(
        uhf.reshape(KP, 2, 128, G).transpose(0, 2, 1, 3)).astype(f8)
    uh2 = (np.ascontiguousarray(
        (uhf + Wd.astype(np.float32) @ Wx.astype(np.float32))
        .reshape(KT, 128, G)).astype(bf) if absorb else None)
    wxa = np.concatenate(
        [Wx.astype(np.float32), b.astype(np.float32)[None, :]], axis=0).astype(bf)
    wd = np.ascontiguousarray(
        Wd.astype(np.float32).reshape(KT, 128, FEAT)).astype(bf)
    bdv = np.ascontiguousarray(bd.astype(np.float32).reshape(FEAT, 1))
    ident = np.eye(128, dtype=np.float32).astype(bf)

    in_maps = []
    for ci in range(N_CORES):
        shard = np.asarray(inputs[ci * B:(ci + 1) * B, :nwarm, :], dtype=np.float32)
        # [B, T, F] -> [T, F, B], then append the ones row -> [T, F+1, B]
        xt = np.transpose(shard, (1, 2, 0))
        xt = np.concatenate([xt, np.ones((nwarm, 1, B), np.float32)], axis=1)
        m = {
            "xt": np.ascontiguousarray(xt).astype(bf),
            "uh": uh, "uh8": uh8, "wxa": wxa, "wd": wd, "bdv": bdv,
            "ident": ident,
        }
        if absorb:
            m["uh2"] = uh2
        in_maps.append(m)
    return in_maps


def run(inputs, Wx, Uh, b, Wd, bd, nwarm=WARM_T, nar=NBLK - 1, trace=False):
    absorb = absorb_ok(Wx, b, bd)
    nc = _get_nc(nwarm, nar, absorb)
    in_maps = prep_inputs(inputs, Wx, Uh, b, Wd, bd, nwarm, absorb=absorb)
    res = run_bass_kernel_spmd(nc, in_maps, core_ids=list(range(N_CORES)),
                               trace=trace)
    outs = []
    for ci in range(N_CORES):
        o = res.results[ci]["out"]          # [nblk, FEAT, B]
        outs.append(np.transpose(o, (2, 0, 1)).reshape(B, (nar + 1) * FEAT, 1))
    full = np.concatenate(outs, axis=0).astype(np.float32)
    return full, res


def kernel(inputs, Wx, Uh, b, Wd, bd):
    full, _ = run(np.asarray(inputs), np.asarray(Wx), np.asarray(Uh),
                  np.asarray(b), np.asarray(Wd), np.asarray(bd))
    return full


if __name__ == "__main__":
    rng = np.random.default_rng(0)
    s = 0.05
    inputs = rng.standard_normal((BATCH, WARM_T, FEAT)).astype(np.float32)
    Wx = (rng.standard_normal((FEAT, G)) * s).astype(np.float32)
    Uh = (rng.standard_normal((UNITS, G)) * s).astype(np.float32)
    b = np.zeros(G, np.float32)
    Wd = (rng.standard_normal((UNITS, FEAT)) * s).astype(np.float32)
    bd = np.zeros(FEAT, np.float32)
    out = kernel(inputs=inputs, Wx=Wx, Uh=Uh, b=b, Wd=Wd, bd=bd)
    print("out shape:", out.shape, out.dtype)


# revision 18
# speedup vs baseline: 1.3817x; 1.0037x over previous
"""AR block LSTM on 8 TRN2 NeuronCores.

Data-parallel over batch (1024 -> 128/core), weights replicated.
Per core, each LSTM step computes z = [x;1] @ [Wx;b] + h @ Uh into 8 PSUM
banks (4096 gate cols, native i|f|g|o order), does the cell math on
ACT/DVE in unit-halves, and transposes h2 back to [units, batch] layout
with PE-transposes so it can serve as the next step's stationary operand.

Precision schedule: the first T8=112 warmup steps run the h @ Uh matmuls
in fp8e4 DoubleRow mode (two k-tiles per matmul, measured 2x PE
throughput); the remaining warmup + all AR steps run bf16.  CPU
simulation shows the LSTM forget gates wash out early-step fp8
quantization noise: end-to-end max-rel error stays near the all-bf16
level as long as the last ~40 steps are bf16.  No operand scaling is
needed at these magnitudes.  Gates and the cell state c are bf16 (DVE
tensor ops hit the 2x packed mode; simulated end-to-end error ~6e-3 vs
the 2e-2 gate); PSUM accumulation stays fp32.

Scheduling: in warm steps the x/bias row-packed matmuls OPEN each PSUM
accumulation group (x is ready early) so each bank closes at its last
k-matmul and the ACT/DVE cell chain starts as early as possible; banks
are processed in chain-dependency order i,g,f,o.  Half-B transposes are
deferred into the next step's matmul stream (pending_tr) for ALL steps
incl. AR, and in the AR phase the p-block (output head + feedback) is
also deferred into the next step's stream, hiding the whole feedback
chain under the next step's k-matmuls.  PSUM: 7 z banks + 1 tr bank.
"""
import os
import numpy as np
import ml_dtypes

import concourse.bass as bass
import concourse.mybir as mybir
import concourse.tile as tile
from concourse import bacc
from concourse.bass_utils import run_bass_kernel_spmd

F32 = mybir.dt.float32
BF16 = mybir.dt.bfloat16
F8 = mybir.dt.float8e4
DR = mybir.MatmulPerfMode.DoubleRow

N_CORES = 8
BATCH = 1024
B = BATCH // N_CORES          # 128 batch rows per core
WARM_T = 128
UNITS = 1024
KT = UNITS // 128             # 8 K-tiles
KP = KT // 2                  # 4 K-tile pairs (DoubleRow)
G = 4 * UNITS                 # 4096 gate cols
NBANK = G // 512              # 8 PSUM banks of 512 gate cols
FEAT = 8
NBLK = 192 // FEAT            # 24 output blocks
CH = 16                       # xT chunk length (warmup steps per DMA)
T8 = 118                      # steps [1, T8) use fp8 DoubleRow k-matmuls

ACT = mybir.ActivationFunctionType


def build(nwarm=WARM_T, nar=NBLK - 1, t8=T8, absorb=True):
    nc = bacc.Bacc("TRN2", target_bir_lowering=False, debug=False,
                   num_devices=N_CORES)
    xt_e = nc.dram_tensor("xt", [nwarm, FEAT + 1, B], BF16, kind="ExternalInput").ap()
    uh_e = nc.dram_tensor("uh", [KT, 128, G], BF16, kind="ExternalInput").ap()
    uh8_e = nc.dram_tensor("uh8", [KP, 128, 2, G], F8, kind="ExternalInput").ap()
    uh2_e = (nc.dram_tensor("uh2", [KT, 128, G], BF16, kind="ExternalInput").ap()
             if absorb else None)
    wxa_e = nc.dram_tensor("wxa", [FEAT + 1, G], BF16, kind="ExternalInput").ap()
    wd_e = nc.dram_tensor("wd", [KT, 128, FEAT], BF16, kind="ExternalInput").ap()
    bd_e = nc.dram_tensor("bdv", [FEAT, 1], F32, kind="ExternalInput").ap()
    id_e = nc.dram_tensor("ident", [128, 128], BF16, kind="ExternalInput").ap()
    out_e = nc.dram_tensor("out", [nar + 1, FEAT, B], F32, kind="ExternalOutput").ap()

    nsteps = nwarm + nar

    with tile.TileContext(nc) as tc:
        with tc.tile_pool(name="w", bufs=1) as wp, \
             tc.tile_pool(name="xt", bufs=2) as xtp, \
             tc.tile_pool(name="ht", bufs=2) as htp, \
             tc.tile_pool(name="h8", bufs=2) as h8p, \
             tc.tile_pool(name="st", bufs=1) as stp, \
             tc.tile_pool(name="h2", bufs=2) as h2p, \
             tc.tile_pool(name="gate", bufs=6) as gp, \
             tc.tile_pool(name="m", bufs=4) as mp, \
             tc.tile_pool(name="pt", bufs=1) as ptp, \
             tc.tile_pool(name="z", bufs=7, space="PSUM") as zp, \
             tc.tile_pool(name="tr", bufs=1, space="PSUM") as trp:

            # ---- resident weights -------------------------------------
            # DMA issue order = priority: the first fp8 step only needs
            # uh8 + chunk0 + wxa; bf16 uh (needed from step T8) and wd
            # (needed from the AR phase) trail.
            wxa_sb = wp.tile([128, G], BF16)
            for r in range(4):
                nc.sync.dma_start(out=wxa_sb[32 * r:32 * r + FEAT + 1, :], in_=wxa_e[:])
            id_sb = wp.tile([128, 128], BF16)
            nc.sync.dma_start(out=id_sb[:], in_=id_e[:])

            # ---- xT chunk prefetch ------------------------------------
            nchunk = (nwarm + CH - 1) // CH
            chunk_tiles = {}

            def load_chunk(ci):
                t0 = ci * CH
                n = min(CH, nwarm - t0)
                tl = xtp.tile([128, CH, B], BF16, tag="xt")
                for r in range(4):
                    nc.sync.dma_start(
                        out=tl[32 * r:32 * r + FEAT + 1, :n, :],
                        in_=xt_e[t0:t0 + n].rearrange("t p b -> p t b"))
                return tl

            if nwarm > 0:
                chunk_tiles[0] = load_chunk(0)

            uh8_sb = []
            for kp in range(KP):
                u8 = wp.tile([128, 2, G], F8, tag=f"uh8{kp}")
                nc.sync.dma_start(out=u8[:], in_=uh8_e[kp])
                uh8_sb.append(u8)

            uh_sb = []
            for k in range(KT):
                u1 = wp.tile([128, G], BF16, tag=f"uh{k}")
                nc.sync.dma_start(out=u1[:], in_=uh_e[k])
                uh_sb.append(u1)
            wd_sb = wp.tile([128, KT, FEAT], BF16)
            for k in range(KT):
                nc.sync.dma_start(out=wd_sb[:, k, :], in_=wd_e[k])
            bd_sb = wp.tile([FEAT, 1], F32)
            nc.sync.dma_start(out=bd_sb[:], in_=bd_e[:])
            uh2_sb = []
            if absorb:
                for k in range(KT):
                    u2 = wp.tile([128, G], BF16, tag=f"uh2{k}")
                    nc.sync.dma_start(out=u2[:], in_=uh2_e[k])
                    uh2_sb.append(u2)

            # ---- state ------------------------------------------------
            hT = h8p.tile([128, KT, B], F8, tag="hT8")
            nc.gpsimd.memset(hT[:], 0.0)
            c_sb = stp.tile([128, UNITS], BF16)
            nc.gpsimd.memset(c_sb[:], 0.0)

            # pta feedback tiles: ones rows written once, p rows per step
            pta_tiles = []
            for i in range(2):
                pt_i = ptp.tile([128, B], BF16, tag=f"pta{i}")
                nc.gpsimd.memset(pt_i[:], 1.0)
                pta_tiles.append(pt_i)

            def p_block(j, hT_new):
                """pT = (h_new @ Wd + bd)^T -> out block j; returns pT_aug."""
                pp = trp.tile([FEAT, B], F32, tag="tr4")
                for k in range(KT):
                    nc.tensor.matmul(pp[:], wd_sb[:, k, :], hT_new[:, k, :],
                                     start=(k == 0), stop=(k == KT - 1))
                pto = ptp.tile([FEAT, B], F32, tag="pto")
                nc.scalar.activation(pto[:], pp[:], ACT.Identity, bias=bd_sb[:])
                nc.sync.dma_start(out=out_e[j], in_=pto[:])
                if absorb:
                    return None
                pta = pta_tiles[j % 2]
                for r in range(4):
                    nc.vector.tensor_copy(pta[32 * r:32 * r + FEAT, :], pto[:])
                return pta

            pta = None
            pend = None       # prev step's half-B transposes
            pend_p = None     # prev step's p-block (AR head + feedback)
            # chain-dependency order: i, g, f, o per half
            bank_order = [0, 4, 2, 6, 1, 5, 3, 7]

            for t in range(nsteps):
                warm = t < nwarm
                use8 = 0 < t < t8
                ab = absorb and not warm
                uh_tbl = uh2_sb if ab else uh_sb
                if warm:
                    ci, s = divmod(t, CH)
                    if s == 0 and ci + 1 < nchunk:
                        chunk_tiles[ci + 1] = load_chunk(ci + 1)
                        chunk_tiles.pop(ci - 1, None)
                    ct = chunk_tiles[ci]
                    xaug = lambda r: ct[32 * r:32 * r + FEAT + 1, s, :]
                else:
                    xaug = lambda r: pta[32 * r:32 * r + FEAT + 1, :]

                zt = {}
                h2 = h2p.tile([128, UNITS], BF16, tag="h2")
                prod8 = (t + 1) < t8   # next step consumes fp8 hT
                if prod8:
                    hT_new = h8p.tile([128, KT, B], F8, tag="hT8")
                else:
                    hT_new = htp.tile([128, KT, B], BF16, tag="hT")

                def half_chain(h):
                    # ACT order matches bank-closing order (i, g, f, o);
                    # DVE ops interleave as their operands become ready.
                    # The half-1 tail (tanh(c), h2 mul) is split in two so
                    # the deferred transposes of k4,5 can start earlier.
                    u0 = h * 512
                    si = gp.tile([128, 512], BF16, tag="g")
                    sf = gp.tile([128, 512], BF16, tag="g")
                    tg = gp.tile([128, 512], BF16, tag="g")
                    so = gp.tile([128, 512], BF16, tag="g")
                    m1 = mp.tile([128, 512], BF16, tag="m")
                    m2 = mp.tile([128, 512], BF16, tag="m")
                    nc.scalar.activation(si[:], zt[0 + h][:], ACT.Sigmoid)
                    nc.scalar.activation(tg[:], zt[4 + h][:], ACT.Tanh)
                    nc.vector.tensor_mul(m2[:], si[:], tg[:])
                    nc.scalar.activation(sf[:], zt[2 + h][:], ACT.Sigmoid)
                    nc.vector.tensor_mul(m1[:], sf[:], c_sb[:, u0:u0 + 512])
                    nc.scalar.activation(so[:], zt[6 + h][:], ACT.Sigmoid)
                    nc.vector.tensor_add(c_sb[:, u0:u0 + 512], m1[:], m2[:])
                    tc2 = gp.tile([128, 512], BF16, tag="g")
                    nc.scalar.activation(tc2[:], c_sb[:, u0:u0 + 512], ACT.Tanh)
                    nc.vector.tensor_mul(h2[:, u0:u0 + 512], so[:], tc2[:])

                def emit_tr(h, h2t, hTn):
                    # 4 transposes into one single-bank PSUM tile, then one
                    # wide copy/cast into the hT layout.  (Splitting this
                    # copy, adding post-pend filler matmuls, or reordering
                    # the consumers all measured SLOWER on silicon: the
                    # changed DVE/PE micro-idle profile trips the HAM clock
                    # gate into 2.4<->1.2 GHz oscillation, ham events 4->60+.)
                    tr = trp.tile([128, 4, 128], BF16, tag="tr4")
                    for kq in range(4):
                        k0 = 4 * h + kq
                        nc.tensor.transpose(tr[:, kq, :],
                                            h2t[:, k0 * 128:(k0 + 1) * 128],
                                            id_sb[:])
                    nc.vector.tensor_copy(hTn[:, 4 * h:4 * h + 4, :], tr[:])

                def kmms(bk, ks, open_grp, close_grp):
                    n0 = bk * 512
                    for j, k in enumerate(ks):
                        nc.tensor.matmul(zt[bk][:], hT[:, k, :],
                                         uh_tbl[k][:, n0:n0 + 512],
                                         start=(open_grp and j == 0),
                                         stop=(close_grp and j == len(ks) - 1))

                def kmms8(bk, kps, open_grp, close_grp):
                    n0 = bk * 512
                    for j, kp in enumerate(kps):
                        nc.tensor.matmul(zt[bk][:], hT[:, 2 * kp:2 * kp + 2, :],
                                         uh8_sb[kp][:, :, n0:n0 + 512],
                                         start=(open_grp and j == 0),
                                         stop=(close_grp and j == len(kps) - 1),
                                         perf_mode=DR)

                def packs(banks, open_grp):
                    # Row-packed x/bias matmuls: K=9 tiles at PE rows
                    # 0/32/64/96 run concurrently, each into its own PSUM
                    # bank. In warm steps x is ready early, so they open
                    # the accumulation groups; in the AR phase x (= pT)
                    # arrives late, so they close them instead.
                    for r, bk in enumerate(banks):
                        n0r = bk * 512
                        nc.tensor.matmul(
                            zt[bk][:],
                            xaug(r),
                            wxa_sb[32 * r:32 * r + FEAT + 1, n0r:n0r + 512],
                            start=open_grp, stop=(t == 0 or not open_grp),
                            tile_position=(32 * r, 0))

                for bk in bank_order:
                    ztile = zp.tile([128, 512], F32, tag="z")
                    zt[bk] = ztile
                bA, bB = bank_order[:4], bank_order[4:]

                # ---- half-A matmul stream --------------------------------
                if warm:
                    packs(bA, open_grp=True)
                if pend is not None:
                    if use8:
                        for bk in bA:
                            kmms8(bk, [0, 1], not warm, False)
                        pend()
                        pend = None
                        for bk in bA:
                            kmms8(bk, [2, 3], False, warm)
                    else:
                        for bk in bA:
                            kmms(bk, [0, 1, 2, 3], not warm, False)
                        pend()
                        pend = None
                        for bk in bA:
                            kmms(bk, [4, 5, 6, 7], False, warm or ab)
                if pend_p is not None:
                    pta = pend_p()
                    pend_p = None
                if not warm and not ab:
                    packs(bA, open_grp=False)
                half_chain(0)

                # ---- half-B matmul stream --------------------------------
                if warm:
                    packs(bB, open_grp=True)
                    if t > 0:
                        if use8:
                            for bk in bB:
                                kmms8(bk, [0, 1, 2, 3], False, True)
                        else:
                            for bk in bB:
                                kmms(bk, list(range(KT)), False, True)
                else:
                    for bk in bB:
                        kmms(bk, list(range(KT)), True, ab)
                    if not ab:
                        packs(bB, open_grp=False)
                emit_tr(0, h2, hT_new)
                half_chain(1)

                if t < nsteps - 1:
                    ch2, chT = h2, hT_new
                    pend = lambda: emit_tr(1, ch2, chT)
                else:
                    emit_tr(1, h2, hT_new)

                hT = hT_new
                if t >= nwarm - 1:
                    if t < nsteps - 1:
                        cj, cht = t - nwarm + 1, hT_new
                        pend_p = lambda: p_block(cj, cht)
                    else:
                        p_block(t - nwarm + 1, hT_new)

    nc.finalize()
    return nc


_NC_CACHE = {}


def _get_nc(nwarm, nar, absorb):
    key = (nwarm, nar, absorb)
    if key not in _NC_CACHE:
        _NC_CACHE[key] = build(nwarm, nar, absorb=absorb)
    return _NC_CACHE[key]


def absorb_ok(Wx, b, bd):
    bp = b.astype(np.float64) + bd.astype(np.float64) @ Wx.astype(np.float64)
    return not np.any(bp)


def prep_inputs(inputs, Wx, Uh, b, Wd, bd, nwarm=WARM_T, absorb=True):
    """Host-side prep: shard + transpose + bf16/fp8. Returns in_maps list."""
    bf = ml_dtypes.bfloat16
    f8 = ml_dtypes.float8_e4m3
    uhf = Uh.astype(np.float32)
    uh = np.ascontiguousarray(uhf.reshape(KT, 128, G)).astype(bf)
    # [KP, 128, 2, G]: uh8[kp][p, i, n] = Uh[(2*kp+i)*128 + p, n]
    uh8 = np.ascontiguousarray(
        uhf.reshape(KP, 2, 128, G).transpose(0, 2, 1, 3)).astype(f8)
    uh2 = (np.ascontiguousarray(
        (uhf + Wd.astype(np.float32) @ Wx.astype(np.float32))
        .reshape(KT, 128, G)).astype(bf) if absorb else None)
    wxa = np.concatenate(
        [Wx.astype(np.float32), b.astype(np.float32)[None, :]], axis=0).astype(bf)
    wd = np.ascontiguousarray(
        Wd.astype(np.float32).reshape(KT, 128, FEAT)).astype(bf)
    bdv = np.ascontiguousarray(bd.astype(np.float32).reshape(FEAT, 1))
    ident = np.eye(128, dtype=np.float32).astype(bf)

    in_maps = []
    for ci in range(N_CORES):
        shard = np.asarray(inputs[ci * B:(ci + 1) * B, :nwarm, :], dtype=np.float32)
        # [B, T, F] -> [T, F, B], then append the ones row -> [T, F+1, B]
        xt = np.transpose(shard, (1, 2, 0))
        xt = np.concatenate([xt, np.ones((nwarm, 1, B), np.float32)], axis=1)
        m = {
            "xt": np.ascontiguousarray(xt).astype(bf),
            "uh": uh, "uh8": uh8, "wxa": wxa, "wd": wd, "bdv": bdv,
            "ident": ident,
        }
        if absorb:
            m["uh2"] = uh2
        in_maps.append(m)
    return in_maps


def run(inputs, Wx, Uh, b, Wd, bd, nwarm=WARM_T, nar=NBLK - 1, trace=False):
    absorb = absorb_ok(Wx, b, bd)
    nc = _get_nc(nwarm, nar, absorb)
    in_maps = prep_inputs(inputs, Wx, Uh, b, Wd, bd, nwarm, absorb=absorb)
    res = run_bass_kernel_spmd(nc, in_maps, core_ids=list(range(N_CORES)),
                               trace=trace)
    outs = []
    for ci in range(N_CORES):
        o = res.results[ci]["out"]          # [nblk, FEAT, B]
        outs.append(np.transpose(o, (2, 0, 1)).reshape(B, (nar + 1) * FEAT, 1))
    full = np.concatenate(outs, axis=0).astype(np.float32)
    return full, res


def kernel(inputs, Wx, Uh, b, Wd, bd):
    full, _ = run(np.asarray(inputs), np.asarray(Wx), np.asarray(Uh),
                  np.asarray(b), np.asarray(Wd), np.asarray(bd))
    return full


if __name__ == "__main__":
    rng = np.random.default_rng(0)
    s = 0.05
    inputs = rng.standard_normal((BATCH, WARM_T, FEAT)).astype(np.float32)
    Wx = (rng.standard_normal((FEAT, G)) * s).astype(np.float32)
    Uh = (rng.standard_normal((UNITS, G)) * s).astype(np.float32)
    b = np.zeros(G, np.float32)
    Wd = (rng.standard_normal((UNITS, FEAT)) * s).astype(np.float32)
    bd = np.zeros(FEAT, np.float32)
    out = kernel(inputs=inputs, Wx=Wx, Uh=Uh, b=b, Wd=Wd, bd=bd)
    print("out shape:", out.shape, out.dtype)
